# revision 6
# baseline (speedup 1.0000x reference)
"""CP-reconstruction contraction (CPRL) Trainium2 kernel.

Math (per the reference):
    W[a,b,c,d] = sum_r w[r] f0[a,r] f1[b,r] f2[c,r] f3[d,r]
    out[x,d]   = sum_{a,b,c} x[x,a,b,c] W[a,b,c,d] + bias

Factored form actually computed (never materializes W):
    Y_x[r, bc]  = sum_a f0[a,r] * x[x,a,bc]            (TensorE matmul)
    t3[x, r]    = sum_bc Y_x[r, bc] * K[r, bc]          (VectorE mul+reduce)
                  where K[r, (b,c)] = f1[b,r] * f2[c,r] (built on device)
    out[x, d]   = sum_r t3[x, r] * w[r] * f3[d,r] + bias (TensorE matmul)

Sharding: data-parallel over batch B=32 -> 4 per core on 8 cores; the small CP
factors are replicated (host-side layout prep only: transpose / zero-pad /
partition-offset tiling).  Each local batch's PSUM rows live at partition
offset 32*b (col-group tile_position), so one VectorE op covers all 4 batches.

Schedule: x streams in 8 sub-chunks of 256 cols on the sync HWDGE ring (FIFO
=> in-order arrival); PE runs a short bf16 warm-up burst during the first DMA
so the HAM clock gate is released before the real fp32 matmuls; VectorE works
at 512-col PSUM-bank granularity behind the PE.
"""

import os
import numpy as np

from concourse import bass, bacc, mybir, tile
from concourse.bass_utils import run_bass_kernel_spmd

# Problem shapes (hardcoded per spec nn_CPRL_24292335026358)
B, D1, D2, D3, O1, R = 32, 128, 64, 32, 64, 16
BC = D2 * D3                # 2048
N_CORES = 8
LB = B // N_CORES           # 4 local batches per core
NSUB = 8
SUB = BC // NSUB            # 256 cols per DMA/matmul sub-chunk
NBANK = 4
BANK = BC // NBANK          # 512 (one PSUM bank of fp32)

F32 = mybir.dt.float32
BF16 = mybir.dt.bfloat16

# Matmul dtype for the big f0 contraction: "float32" (exact, 4 cyc/row) or
# "float32r" (1 cyc/row at N>=256, ~1.5e-4 relaxed numerics).
MM_DT = getattr(mybir.dt, os.environ.get("CPRL_MM_DT", "float32"))
WARM_MMS = int(os.environ.get("CPRL_WARM_MMS", "10"))

_CACHE = {}


def _build_program(unroll=1):
    key = (str(MM_DT), unroll, WARM_MMS)
    if key in _CACHE:
        return _CACHE[key]

    nc = bacc.Bacc(
        "TRN2", target_bir_lowering=False, debug=False, enable_asserts=False
    )

    x_d = nc.dram_tensor("x", [LB, D1, BC], F32, kind="ExternalInput")
    f0_d = nc.dram_tensor("f0p", [D1, 32], F32, kind="ExternalInput")
    c2_d = nc.dram_tensor("consts", [128, 162], F32, kind="ExternalInput")
    out_d = nc.dram_tensor("out", [LB, O1], F32, kind="ExternalOutput")

    is_f32r = MM_DT == mybir.dt.float32r
    x_dma = nc.gpsimd.dma_start if is_f32r else nc.sync.dma_start

    with tile.TileContext(nc) as tc:
        with (
            tc.tile_pool(name="const", bufs=1) as cpool,
            tc.tile_pool(name="xp", bufs=NSUB) as xpool,
            tc.tile_pool(name="scr", bufs=2) as spool,
            tc.tile_pool(name="ps", bufs=1, space=bass.MemorySpace.PSUM) as pspool,
        ):
            # ---- PE warm-up: release the HAM clock gate during first DMAs ----
            if WARM_MMS:
                wsb = cpool.tile([128, 256], BF16)
                wps = pspool.tile([16, 256], F32, tag="warm")
                nc.vector.memset(wsb[:, :], 0.0)
                for _ in range(WARM_MMS):
                    nc.tensor.matmul(
                        wps[:, :], wsb[:, 0:16], wsb[:, :], start=True, stop=True
                    )

            # ---- constants ----
            f0t = cpool.tile([D1, 32], MM_DT)   # [a, m] m: 0:16=f0, 16:32=0
            c2 = cpool.tile([128, 162], F32)    # f1T|f2T|f3T|w|bias tiled at 32*g
            kt = cpool.tile([128, BC], F32)     # K^T, rows 32g+[0:16] live

            x_dma(f0t[:, :], f0_d.ap())
            nc.scalar.dma_start(c2[:, :], c2_d.ap())

            f1s = c2[:, 0:64]         # f1T tiled
            f2s = c2[:, 64:96]        # f2T tiled
            f3s = c2[:, 96:160]       # f3T tiled
            ws = c2[:, 160:161]       # w tiled
            bs = c2[:, 161:162]       # bias everywhere

            # K^T[p, (b2,c)] = f1T[p,b2] * f2T[p,c]  (zero rows stay zero)
            in0, in1 = bass.broadcast_tensor_aps(
                f1s.unsqueeze(2), f2s.unsqueeze(1)
            )
            nc.vector.tensor_tensor(
                kt[:, :].rearrange("p (b c) -> p b c", c=D3),
                in0,
                in1,
                op=mybir.AluOpType.mult,
            )

            # F3w^T[p, d] = f3T[p, d] * w[p]
            f3wt = cpool.tile([128, O1], F32)
            nc.vector.tensor_scalar_mul(f3wt[:, :], f3s, ws)

            # ---- main pipeline (unroll>1 only for benchmarking) ----
            xin = x_d.ap().transpose([1, 0, 2])  # [D1, LB, BC]
            for _ in range(unroll):
                y = pspool.tile([128, BC], F32, tag="y")    # 4 banks
                po = pspool.tile([128, O1], F32, tag="po")  # 1 bank
                t3p = cpool.tile([128, NBANK], F32, tag="t3p")
                t3 = cpool.tile([128, 1], F32, tag="t3")
                osb = cpool.tile([128, O1], F32, tag="osb")
                for s in range(NSUB):
                    xt = xpool.tile([D1, LB, SUB], MM_DT, tag="xt")
                    x_dma(xt[:, :, :], xin[:, :, bass.ts(s, SUB)])
                    for b in range(LB):
                        nc.tensor.matmul(
                            y[32 * b : 32 * b + 32, bass.ts(s, SUB)],
                            f0t[:, :],
                            xt[:, b, :],
                            start=True,
                            stop=True,
                            tile_position=(0, 32 * b),
                        )
                    if s % 2 == 1:
                        j = s // 2
                        scr = spool.tile([128, BANK], F32, tag="scr")
                        nc.vector.tensor_tensor(
                            scr[:, :],
                            y[:, bass.ts(j, BANK)],
                            kt[:, bass.ts(j, BANK)],
                            op=mybir.AluOpType.mult,
                        )
                        nc.vector.tensor_reduce(
                            t3p[:, j : j + 1], scr[:, :],
                            axis=mybir.AxisListType.X,
                            op=mybir.AluOpType.add,
                        )

                nc.vector.tensor_reduce(
                    t3[:, 0:1], t3p[:, :], axis=mybir.AxisListType.X,
                    op=mybir.AluOpType.add,
                )

                # out[b, d] = sum_r t3[32b+r] * f3w[32b+r, d] + bias
                for g in range(LB):
                    p = 32 * g
                    nc.tensor.matmul(
                        po[p : p + 1, :],
                        t3[p : p + 16, 0:1],
                        f3wt[p : p + 16, :],
                        start=True,
                        stop=True,
                        tile_position=(p, p),
                    )
                    nc.vector.tensor_scalar_add(
                        osb[p : p + 1, :], po[p : p + 1, :], bs[p : p + 1, 0:1]
                    )
                nc.sync.dma_start(out_d.ap()[:, :], osb[0 : 32 * (LB - 1) + 1 : 32, :])

    nc.compile()
    _CACHE[key] = nc
    return nc


def _prep_consts(f0, f1, f2, f3, cp_weights, bias):
    """Layout-only host prep: transpose / zero-pad / tile at partition offsets."""
    f0p = np.zeros((D1, 32), np.float32)
    f0p[:, :R] = f0
    c2 = np.zeros((128, 162), np.float32)
    for g in range(LB):
        r0 = 32 * g
        c2[r0 : r0 + R, 0:64] = f1.T
        c2[r0 : r0 + R, 64:96] = f2.T
        c2[r0 : r0 + R, 96:160] = f3.T
        c2[r0 : r0 + R, 160] = cp_weights
    c2[:, 161] = bias[0]
    return f0p, c2


def kernel(x, f0, f1, f2, f3, cp_weights, bias):
    nc = _build_program()
    f0p, c2 = _prep_consts(
        np.asarray(f0, np.float32),
        np.asarray(f1, np.float32),
        np.asarray(f2, np.float32),
        np.asarray(f3, np.float32),
        np.asarray(cp_weights, np.float32),
        np.asarray(bias, np.float32),
    )
    xr = np.ascontiguousarray(np.asarray(x, np.float32)).reshape(B, D1, BC)

    in_maps = []
    for i in range(N_CORES):
        in_maps.append(
            {
                "x": np.ascontiguousarray(xr[LB * i : LB * (i + 1)]),
                "f0p": f0p,
                "consts": c2,
            }
        )

    res = run_bass_kernel_spmd(nc, in_maps, core_ids=list(range(N_CORES)))
    out = np.concatenate([r["out"] for r in res.results], axis=0)
    return out.astype(np.float32)
